# revision 4
# baseline (speedup 1.0000x reference)
"""ConvTranspose1d (B=16, Cin=Cout=64, K=8, L=32768, stride=1) on 8 trn2 cores.

fp8-DoubleRow rewrite of the f32r baseline: 99907 ns -> 68757 ns modeled
(TimelineSim instruction-cost model), rel err 2.5e-3 vs the 2e-2 gate.

Precision: x and w are each split hi/lo in e4m3; the device computes
(x8h + x8l) * (w8h + w8l) exactly (residuals ~0.1%); output is bf16.

Layout: each core's 2 batches are host-interleaved along columns
(col 2t+b), so one matmul/epilogue stream serves both. Contraction uses
128 partitions = (hi|lo, cin) - both halves DMA'd directly, no on-chip
shift copies. DoubleRow's second k-tile dim (kt, 0.5 PE cyc/col,
256-slot contraction) carries adjacent taps via an overlapping strided
SBUF access pattern (kt stride 2 = 1 position).

Per chunk (252 positions x 2 batches, psum [128, 512], 1 bank): 4 DR
matmuls (w_hi/w_lo x m in {0,1}); taps j = 4h+2m+kt' with h the psum
partition-half split: P[(h,o), 2i+b] = C_h (taps [4h,4h+4)). 2.03 PE
cyc per output position-batch, 100% MAC utilization (~56us at 2.4GHz).

Epilogue per pair of chunks ([128, 1024] 2-bank psum, supers of sg=2
pairs share one ev tile), split two ways per a deterministic ratio:
 - device-combine (65%): heavy = ONE [128, w] PSUM->SBUF bf16 pass
   (engine cost scales with columns only, so one pass evacuates BOTH
   h halves; bias fused via bias128 zeroed on h=0 rows), on ACT 83% /
   DVE 17%; c1 = cross-base copy of the C_1 half to base partition 0
   (2-input ops cannot mix SBUF base partitions; 1-input copies can),
   DVE 4x-mode 65% / Pool 35%; c2 = in-place all-bf16 DVE 2x add
   tm += ev[h=0 rows, +8 cols]; one strided store.
 - host-combine (35%): after the heavy pass, store BOTH ev halves (out
   + aux dram) and let the host add them - trades scarce ACT/DVE/Pool
   cycles for idle DMA bandwidth. hc column ranges are recorded at
   build time and consumed in run_on_hw.
Tail supers force device-combine with c1 on DVE (shortest drain chain);
window loads prefetch 2 ahead; stores issue from the SP queue only
(ACT-issued DMAs stall the heavy stream; HWDGE costs ~625ns/DMA on a
serialized resource, so DMA instruction count is minimized throughout:
1 load/window, 1-2 stores/super).

Host side: e4m3 hi/lo split + batch interleave of x (zero-padded
margins), stationary pack of w (both hl halves identical per G block),
bias128, bf16->f32 convert, hc-range adds, de-interleave. Modeled
resource busy per core: PE 55.9us, ACT ~57, DMA engines ~55, DVE ~45,
Pool ~32; wall 68.8us = busy + ~3us start + ~5us drain.
"""

import sys

sys.path.insert(0, "/opt/trn_rl_repo")

import numpy as np
import ml_dtypes

import concourse.bass as bass
import concourse.tile as tile
from concourse import bacc, mybir
from concourse import bass_utils

B, CIN, COUT, KW, L = 16, 64, 64, 8, 32768
NCORES = 8
BPC = B // NCORES
LOUT = L + KW - 1
NMM = 512          # psum bank width (f32 cols) = matmul max free size
NPOS = (NMM - 8) // 2  # output positions (per batch) per chunk = 252
PAD = 8            # zero positions padded on each side of x (host)
F32 = mybir.dt.float32
BF16 = mybir.dt.bfloat16
F8 = mybir.dt.float8e4
E4M3 = ml_dtypes.float8_e4m3
DR = mybir.MatmulPerfMode.DoubleRow
AF = mybir.ActivationFunctionType
ADD = mybir.AluOpType.add


def _win_schedule(nchunks, ramp, steady, tail_ramp=()):
    tail = list(tail_ramp)
    while tail and nchunks - sum(tail) < sum(ramp):
        tail.pop(0)
    body = nchunks - sum(tail)
    sched = []
    for r in ramp:
        if sum(sched) + r > body:
            break
        sched.append(r)
    while sum(sched) < body:
        sched.append(min(steady, body - sum(sched)))
    sched += tail
    return sched


def _slide3(xd, base, n):
    """[128(or 64), 2, n] view of 2D tile xd with kt stride 2, col stride 1,
    starting at column `base` (overlapping windows for DoubleRow)."""
    v = xd[:, base : base + 4].rearrange("p (a b) -> p a b", a=2).copy()
    ap = v.ap
    ap[1] = [2, 2]
    ap[2] = [1, n]
    v.ap = ap
    return v


def _pairview(ev, p0, p1, base, n, gstride, g=2):
    """[p0:p1, g, n] view of tile ev: dim1 stride gstride (chunk index),
    dim2 stride 1, starting at column base."""
    v = ev[p0:p1, base : base + 2].rearrange("p (a b) -> p a b", a=2).copy()
    ap = v.ap
    ap[1] = [gstride, g]
    ap[2] = [1, n]
    v.ap = ap
    return v


def build(
    nc,
    l=L,
    steady_win=16,
    ramp=(2, 4, 8),
    xd_bufs=4,
    ps_bufs=4,
    ev_bufs=6,
    ob_bufs=4,
    heavy_fracs=(("act", 0.83), ("dve", 0.17)),
    c1_fracs=(("dve", 0.65), ("pool", 0.35)),
    prefetch=2,
    sg=2,
    tail_ramp=(),
    consts_first=True,
    taper_lag0=False,
    taper_wins=1,
    taper_hc=False,
    st_fracs=(("sp", 1.0),),
    hc_frac=0.35,
    dropb_frac=0.0,
    hc_ranges=None,
):
    lout = l + KW - 1
    xx = nc.dram_tensor("xx", [128, 2 * (l + 2 * PAD)], F8, kind="ExternalInput")
    wt = nc.dram_tensor("wt", [128, 4 * 256], F8, kind="ExternalInput")
    bi = nc.dram_tensor("bi", [128, 1], F32, kind="ExternalInput")
    out = nc.dram_tensor("out", [COUT, 2 * lout], BF16, kind="ExternalOutput")
    aux = nc.dram_tensor("aux", [COUT, 2 * lout], BF16, kind="ExternalOutput")
    xxap, wap, bap, oap = xx.ap(), wt.ap(), bi.ap(), out.ap()
    auxap = aux.ap()
    if hc_ranges is None:
        hc_ranges = []

    # chunk k: emits positions [e0p, e0p+ne) for both batches
    nchunks = -(-lout // NPOS)
    chunks = []
    for k in range(nchunks):
        e0p = k * NPOS
        ne = min(NPOS, lout - e0p)
        t0 = e0p - 4
        n_mm = 8 + 2 * ne
        chunks.append((t0, e0p, ne, n_mm))
    wins = []
    i = 0
    for w in _win_schedule(nchunks, ramp, steady_win, tail_ramp):
        wins.append(chunks[i : i + w])
        i += w

    with tile.TileContext(nc) as tc:
        with (
            tc.tile_pool(name="const", bufs=1) as constp,
            tc.tile_pool(name="xd", bufs=xd_bufs) as xdp,
            tc.tile_pool(name="ev", bufs=ev_bufs) as evp,
            tc.tile_pool(name="outp", bufs=ob_bufs) as outp,
            tc.tile_pool(name="psum", bufs=ps_bufs, space=bass.MemorySpace.PSUM) as psp,
        ):
            wt_sb = constp.tile([128, 4 * 256], F8, tag="wt")
            bi_sb = constp.tile([128, 1], F32, tag="bi")
            warm = constp.tile([128, 1], F32, tag="warm")

            def emit_consts():
                nc.sync.dma_start(wt_sb[:], wap[:])
                nc.gpsimd.dma_start(bi_sb[:], bap[:])
                # warm the ACT Identity table before the first activation
                nc.scalar.activation(warm[:], bi_sb[:], AF.Identity, bias=0.0)

            def emit_loads(win):
                s0 = win[0][0] - 3  # position of xd col pair 0
                wspan = max(2 * (t0 - s0) + n_mm for (t0, _, _, n_mm) in win)
                # +4 tile margin for the _slide3 slicing helper; never loaded
                # nor read by the matmul access patterns.
                xd = xdp.tile([128, wspan + 4], F8, tag="xd")
                c0 = 2 * (s0 + PAD)
                nc.sync.dma_start(xd[:, 0:wspan], xxap[:, c0 : c0 + wspan])
                return s0, xd

            def _sched(fracs):
                accs = [0.0] * len(fracs)

                def pick():
                    best, bi_ = None, 0
                    for i, (eng, f) in enumerate(fracs):
                        accs[i] += f
                        if best is None or accs[i] > best:
                            best, bi_ = accs[i], i
                    accs[bi_] -= 1.0
                    return fracs[bi_][0]

                return pick

            pick_heavy = _sched(heavy_fracs)
            pick_c1 = _sched(c1_fracs)
            pick_hc = _sched((("hc", hc_frac), ("dev", 1.0 - hc_frac)))
            pick_st = _sched(st_fracs)
            pick_db = _sched((("drop", dropb_frac), ("keep", 1.0 - dropb_frac)))
            st_engs = {"sp": nc.sync, "act": nc.scalar}

            def st_dma(dst, srcv):
                st_engs[pick_st()].dma_start(dst, srcv)
            # software-pipelined epilogue: c1/c2/store of group i are emitted
            # after group i+lag's matmuls+heavy, so the DVE/Pool queues never
            # head-of-line block on a heavy pass that is still in flight.
            pending = []

            SGW = sg * 2 * NMM
            curS = []  # [(grp, ev, evoff, wtot)] accumulating full pairs

            def emit_mm_group(grp, s0, xd):
                ps = psp.tile([128, 2 * NMM], F32, tag="ps", name="ps")
                for gi, (t0, e0p, ne, n_mm) in enumerate(grp):
                    go = gi * NMM
                    # optionally skip the w_lo correction matmuls (G=1) on a
                    # fraction of chunks: trades ~0.9-1.3% extra rel err for
                    # 2 of 4 matmuls on those chunks
                    nj = 2 if pick_db() == "drop" else 4
                    for j, (g, m) in enumerate(
                        ((0, 0), (0, 1), (1, 0), (1, 1))[:nj]
                    ):
                        base = 2 * (t0 - s0 - 2 * m - 1)
                        lw = wt_sb[
                            :, (2 * g + m) * 256 : (2 * g + m + 1) * 256
                        ].rearrange("p (a q) -> p a q", a=2)
                        nc.tensor.matmul(
                            ps[:, go : go + n_mm],
                            lw,
                            _slide3(xd, base, n_mm),
                            start=(j == 0),
                            stop=(j == nj - 1),
                            perf_mode=DR,
                        )
                return ps

            def emit_heavy(ps, ev, evoff, wtot):
                # heavy: ONE [128, wtot] PSUM->SBUF bf16 pass, bias fused
                # (bias128 is zero on the h=0 rows so it lands once).
                he = pick_heavy()
                if he == "act":
                    nc.scalar.activation(
                        ev[:, evoff : evoff + wtot], ps[:, 0:wtot],
                        AF.Identity, bias=bi_sb[:, 0:1],
                    )
                else:
                    nc.vector.tensor_scalar_add(
                        ev[:, evoff : evoff + wtot], ps[:, 0:wtot],
                        bi_sb[:, 0:1],
                    )

            LASTQ = 2 * (nchunks - 5) * NPOS

            def phase2(sup, force_hc=False):
                ev = sup[0][1]
                wall = sup[-1][2] + sup[-1][3]
                is_tail = 2 * sup[-1][0][-1][1] >= LASTQ
                chunksA = [
                    (evoff // NMM + gi, g)
                    for grp, _, evoff, _ in sup
                    for gi, g in enumerate(grp)
                ]
                if (force_hc or pick_hc() == "hc") and not is_tail:
                    # host-combined super: no on-chip h-add; store the
                    # C_1(+bias) half to out and the shifted C_0 half to aux;
                    # the host adds them (engine work traded for idle DMA).
                    nf = sum(1 for _, g in chunksA if g[2] == NPOS)
                    if nf:
                        qq = 2 * NPOS
                        e0q0 = 2 * sup[0][0][0][1]
                        od = oap[:, e0q0 : e0q0 + nf * qq].rearrange(
                            "p (g q) -> p g q", g=nf
                        )
                        st_dma(od, _pairview(ev, 64, 128, 0, qq, NMM, nf))
                        ad = auxap[:, e0q0 : e0q0 + nf * qq].rearrange(
                            "p (g q) -> p g q", g=nf
                        )
                        st_dma(ad, _pairview(ev, 0, 64, 8, qq, NMM, nf))
                        hc_ranges.append((e0q0, nf * qq))
                    for gidx, (t0, e0p, ne, n_mm) in chunksA[nf:]:
                        qq = 2 * ne
                        st_dma(
                            oap[:, 2 * e0p : 2 * e0p + qq],
                            ev[64:128, gidx * NMM : gidx * NMM + qq],
                        )
                        st_dma(
                            auxap[:, 2 * e0p : 2 * e0p + qq],
                            ev[0:64, gidx * NMM + 8 : gidx * NMM + 8 + qq],
                        )
                        hc_ranges.append((2 * e0p, qq))
                    return
                # cheap1: cross-base copy of the C_1(+bias) half to base
                # partition 0 (2-input ops may not cross SBUF bases); one op
                # spanning the whole super-group's ev tile.
                tm = outp.tile([64, SGW], BF16, tag="tm")
                c1 = "dve" if is_tail else pick_c1()
                if c1 == "dve":
                    nc.vector.tensor_copy(tm[:, 0:wall], ev[64:128, 0:wall])
                elif c1 == "pool":
                    nc.gpsimd.tensor_copy(tm[:, 0:wall], ev[64:128, 0:wall])
                else:
                    nc.scalar.activation(
                        tm[:, 0:wall], ev[64:128, 0:wall], AF.Identity,
                        bias=0.0,
                    )
                # cheap2 (in-place, all-SBUF, base-aligned):
                #   tm[o, (chunk, q)] += ev[o (h=0), (chunk, q+8)]
                chunks_ = chunksA
                nfull = sum(1 for _, g in chunks_ if g[2] == NPOS)
                assert all(g[2] == NPOS for _, g in chunks_[:nfull])
                if nfull:
                    qq = 2 * NPOS
                    o3 = _pairview(tm, 0, 64, 0, qq, NMM, nfull)
                    i0 = _pairview(ev, 0, 64, 8, qq, NMM, nfull)
                    nc.vector.tensor_tensor(o3, o3, i0, ADD)
                for gidx, (t0, e0p, ne, n_mm) in chunks_[nfull:]:
                    qq = 2 * ne
                    tv = tm[:, gidx * NMM : gidx * NMM + qq]
                    nc.vector.tensor_tensor(
                        tv, tv,
                        ev[0:64, gidx * NMM + 8 : gidx * NMM + 8 + qq], ADD,
                    )
                # store: full chunks in one strided DMA; ragged tails alone
                if nfull:
                    qq = 2 * NPOS
                    ost = _pairview(tm, 0, 64, 0, qq, NMM, nfull)
                    e0q0 = 2 * sup[0][0][0][1]
                    od = oap[:, e0q0 : e0q0 + nfull * qq].rearrange(
                        "p (g q) -> p g q", g=nfull
                    )
                    st_dma(od, ost)
                for gidx, (t0, e0p, ne, n_mm) in chunks_[nfull:]:
                    qq = 2 * ne
                    st_dma(
                        oap[:, 2 * e0p : 2 * e0p + qq],
                        tm[:, gidx * NMM : gidx * NMM + qq],
                    )

            def flush_super():
                if curS:
                    pending.append(list(curS))
                    curS.clear()

            ntail = [0]

            def emit_chunks(win, s0, xd, taper=False):
                # pair adjacent full chunks; leftovers go alone
                groups = []
                ci = 0
                while ci < len(win):
                    grp = [win[ci]]
                    ci += 1
                    if (
                        ci < len(win)
                        and grp[0][3] == NMM
                        and win[ci][3] == NMM
                    ):
                        grp.append(win[ci])
                        ci += 1
                    groups.append(grp)
                for grp in groups:
                    ps = emit_mm_group(grp, s0, xd)
                    wtot = (len(grp) - 1) * NMM + grp[-1][3]
                    full_pair = wtot == 2 * NMM
                    if not full_pair:
                        flush_super()
                    ev = (
                        curS[0][1]
                        if curS
                        else evp.tile([128, SGW], BF16, tag="ev")
                    )
                    evoff = curS[-1][2] + 2 * NMM if curS else 0
                    emit_heavy(ps, ev, evoff, wtot)
                    curS.append((grp, ev, evoff, wtot))
                    if not full_pair or len(curS) >= (1 if taper else sg):
                        flush_super()
                    lag = 0 if (taper and taper_lag0) else 1
                    while len(pending) > lag:
                        phase2(pending.pop(0), force_hc=(taper and taper_hc))

            if consts_first:
                emit_consts()
                loaded = [emit_loads(wins[0])]
            else:
                loaded = [emit_loads(wins[0])]
                emit_consts()
            for i, win in enumerate(wins):
                pf = 1 if i == 0 else prefetch
                for j in range(i + 1, min(i + 1 + pf, len(wins))):
                    if j == len(loaded):
                        loaded.append(emit_loads(wins[j]))
                tw = taper_wins if taper_wins is not None else max(
                    1, len(tail_ramp)
                )
                emit_chunks(win, *loaded[i], taper=(i >= len(wins) - tw))
            flush_super()
            for sup in pending:
                phase2(sup, force_hc=taper_hc)
    return xx, wt, bi, out, aux


def pack_x_core(xc, l=L):
    """xc: [2, CIN, l] f32 -> [128, 2*(l+2*PAD)] e4m3: rows 0:64 = e4m3
    hi part, rows 64:128 = e4m3 of the residual; batch-interleaved cols
    (col 2*(t+PAD)+b) with zero margins."""
    x8h = xc.astype(E4M3)
    x8l = (xc - x8h.astype(np.float32)).astype(E4M3)
    arr = np.zeros((128, 2 * (l + 2 * PAD)), dtype=E4M3)
    for r, x8 in ((0, x8h), (64, x8l)):
        v = arr[r : r + CIN, 2 * PAD : 2 * (PAD + l)].reshape(CIN, l, 2)
        v[:, :, 0] = x8[0]
        v[:, :, 1] = x8[1]
    return arr


def pack_weight(weight):
    """[COUT, CIN, KW] f32 -> [128, 1024] e4m3 stationary blocks.
    Block (g, m) cols = (kt, h, o); value w_g[o, c, 4h + 2m + (1-kt)];
    rows = (hl, c) with both hl halves identical."""
    w = np.asarray(weight, dtype=np.float32)
    w8h = w.astype(E4M3)
    w8l = (w - w8h.astype(np.float32)).astype(E4M3)
    blocks = []
    for wg in (w8h, w8l):
        wgf = wg.astype(np.float32)
        for m in range(2):
            blk = np.empty((CIN, 2, 2, COUT), dtype=np.float32)
            for kt in range(2):
                for h in range(2):
                    j = 4 * h + 2 * m + (1 - kt)
                    blk[:, kt, h, :] = wgf[:, :, j].T  # [c, o]
            blocks.append(blk.reshape(CIN, 256))
    half = np.concatenate(blocks, axis=1)  # [64, 1024]
    return np.concatenate([half, half], axis=0).astype(E4M3)


def pack_bias(bias):
    b = np.zeros((128, 1), dtype=np.float32)
    b[64:128, 0] = np.asarray(bias, dtype=np.float32)
    return b


_CACHE = {}


def _compiled():
    if "nc" not in _CACHE:
        nc = bacc.Bacc(
            "TRN2", target_bir_lowering=False, debug=False, num_devices=NCORES
        )
        hc_ranges = []
        handles = build(nc, hc_ranges=hc_ranges)
        nc.compile()
        _CACHE["nc"] = nc
        _CACHE["names"] = [h.name for h in handles]
        _CACHE["hc"] = hc_ranges
    return _CACHE["nc"], _CACHE["names"], _CACHE["hc"]


def run_on_hw(x, weight, bias, trace=False, **kw):
    nc, (xxn, wn, bn, on, an), hc_ranges = _compiled()
    wt_p = pack_weight(weight)
    bi_p = pack_bias(bias)
    x = np.asarray(x, dtype=np.float32)
    in_maps = []
    for k in range(NCORES):
        xx_p = pack_x_core(x[BPC * k : BPC * (k + 1)])
        in_maps.append({xxn: xx_p, wn: wt_p, bn: bi_p})
    res = bass_utils.run_bass_kernel_spmd(
        nc, in_maps, core_ids=list(range(NCORES)), trace=trace, **kw
    )
    outs = []
    for k in range(NCORES):
        oi = np.asarray(res.results[k][on]).astype(np.float32)  # [64, 2*LOUT]
        ai = np.asarray(res.results[k][an])
        for q0, qn in hc_ranges:
            oi[:, q0 : q0 + qn] += ai[:, q0 : q0 + qn].astype(np.float32)
        oi = oi.reshape(COUT, LOUT, 2)
        outs.append(np.stack([oi[:, :, 0], oi[:, :, 1]], axis=0))
    return np.concatenate(outs, axis=0), res


def kernel(x, weight, bias):
    out, _ = run_on_hw(x, weight, bias, trace=False)
    return out


# revision 5
# speedup vs baseline: 1.0347x; 1.0347x over previous
"""ConvTranspose1d (B=16, Cin=Cout=64, K=8, L=32768, stride=1) on 8 trn2 cores.

fp8-DoubleRow rewrite of the f32r baseline: 99907 ns -> 68757 ns modeled
(TimelineSim instruction-cost model), rel err 2.5e-3 vs the 2e-2 gate.

Precision: x and w are each split hi/lo in e4m3; the device computes
(x8h + x8l) * (w8h + w8l) exactly (residuals ~0.1%); output is bf16.

Layout: each core's 2 batches are host-interleaved along columns
(col 2t+b), so one matmul/epilogue stream serves both. Contraction uses
128 partitions = (hi|lo, cin) - both halves DMA'd directly, no on-chip
shift copies. DoubleRow's second k-tile dim (kt, 0.5 PE cyc/col,
256-slot contraction) carries adjacent taps via an overlapping strided
SBUF access pattern (kt stride 2 = 1 position).

Per chunk (252 positions x 2 batches, psum [128, 512], 1 bank): 4 DR
matmuls (w_hi/w_lo x m in {0,1}); taps j = 4h+2m+kt' with h the psum
partition-half split: P[(h,o), 2i+b] = C_h (taps [4h,4h+4)). 2.03 PE
cyc per output position-batch, 100% MAC utilization (~56us at 2.4GHz).

Epilogue per pair of chunks ([128, 1024] 2-bank psum, supers of sg=2
pairs share one ev tile), split two ways per a deterministic ratio:
 - device-combine (65%): heavy = ONE [128, w] PSUM->SBUF bf16 pass
   (engine cost scales with columns only, so one pass evacuates BOTH
   h halves; bias fused via bias128 zeroed on h=0 rows), on ACT 83% /
   DVE 17%; c1 = cross-base copy of the C_1 half to base partition 0
   (2-input ops cannot mix SBUF base partitions; 1-input copies can),
   DVE 4x-mode 65% / Pool 35%; c2 = in-place all-bf16 DVE 2x add
   tm += ev[h=0 rows, +8 cols]; one strided store.
 - host-combine (35%): after the heavy pass, store BOTH ev halves (out
   + aux dram) and let the host add them - trades scarce ACT/DVE/Pool
   cycles for idle DMA bandwidth. hc column ranges are recorded at
   build time and consumed in run_on_hw.
Tail supers force device-combine with c1 on DVE (shortest drain chain);
window loads prefetch 2 ahead; stores issue from the SP queue only
(ACT-issued DMAs stall the heavy stream; HWDGE costs ~625ns/DMA on a
serialized resource, so DMA instruction count is minimized throughout:
1 load/window, 1-2 stores/super).

Host side: e4m3 hi/lo split + batch interleave of x (zero-padded
margins), stationary pack of w (both hl halves identical per G block),
bias128, bf16->f32 convert, hc-range adds, de-interleave. Modeled
resource busy per core: PE 55.9us, ACT ~57, DMA engines ~55, DVE ~45,
Pool ~32; wall 68.8us = busy + ~3us start + ~5us drain.
"""

import sys

sys.path.insert(0, "/opt/trn_rl_repo")

import numpy as np
import ml_dtypes

import concourse.bass as bass
import concourse.tile as tile
from concourse import bacc, mybir
from concourse import bass_utils

B, CIN, COUT, KW, L = 16, 64, 64, 8, 32768
NCORES = 8
BPC = B // NCORES
LOUT = L + KW - 1
NMM = 512          # psum bank width (f32 cols) = matmul max free size
NPOS = (NMM - 8) // 2  # output positions (per batch) per chunk = 252
PAD = 8            # zero positions padded on each side of x (host)
F32 = mybir.dt.float32
BF16 = mybir.dt.bfloat16
F8 = mybir.dt.float8e4
E4M3 = ml_dtypes.float8_e4m3
DR = mybir.MatmulPerfMode.DoubleRow
AF = mybir.ActivationFunctionType
ADD = mybir.AluOpType.add


def _win_schedule(nchunks, ramp, steady, tail_ramp=()):
    tail = list(tail_ramp)
    while tail and nchunks - sum(tail) < sum(ramp):
        tail.pop(0)
    body = nchunks - sum(tail)
    sched = []
    for r in ramp:
        if sum(sched) + r > body:
            break
        sched.append(r)
    while sum(sched) < body:
        sched.append(min(steady, body - sum(sched)))
    sched += tail
    return sched


def _slide3(xd, base, n):
    """[128(or 64), 2, n] view of 2D tile xd with kt stride 2, col stride 1,
    starting at column `base` (overlapping windows for DoubleRow)."""
    v = xd[:, base : base + 4].rearrange("p (a b) -> p a b", a=2).copy()
    ap = v.ap
    ap[1] = [2, 2]
    ap[2] = [1, n]
    v.ap = ap
    return v


def _pairview(ev, p0, p1, base, n, gstride, g=2):
    """[p0:p1, g, n] view of tile ev: dim1 stride gstride (chunk index),
    dim2 stride 1, starting at column base."""
    v = ev[p0:p1, base : base + 2].rearrange("p (a b) -> p a b", a=2).copy()
    ap = v.ap
    ap[1] = [gstride, g]
    ap[2] = [1, n]
    v.ap = ap
    return v


def build(
    nc,
    l=L,
    steady_win=16,
    ramp=(2, 4, 8),
    xd_bufs=4,
    ps_bufs=4,
    ev_bufs=7,
    ob_bufs=4,
    heavy_fracs=(("act", 0.75), ("dve", 0.25)),
    c1_fracs=(("dve", 0.65), ("pool", 0.35)),
    prefetch=2,
    sg=2,
    tail_ramp=(),
    consts_first=True,
    taper_lag0=False,
    taper_wins=1,
    taper_hc=False,
    st_fracs=(("sp", 1.0),),
    hc_frac=0.42,
    dropb_frac=0.25,
    hc_ranges=None,
):
    lout = l + KW - 1
    xx = nc.dram_tensor("xx", [128, 2 * (l + 2 * PAD)], F8, kind="ExternalInput")
    wt = nc.dram_tensor("wt", [128, 4 * 256], F8, kind="ExternalInput")
    bi = nc.dram_tensor("bi", [128, 1], F32, kind="ExternalInput")
    out = nc.dram_tensor("out", [COUT, 2 * lout], BF16, kind="ExternalOutput")
    aux = nc.dram_tensor("aux", [COUT, 2 * lout], BF16, kind="ExternalOutput")
    xxap, wap, bap, oap = xx.ap(), wt.ap(), bi.ap(), out.ap()
    auxap = aux.ap()
    if hc_ranges is None:
        hc_ranges = []

    # chunk k: emits positions [e0p, e0p+ne) for both batches
    nchunks = -(-lout // NPOS)
    chunks = []
    for k in range(nchunks):
        e0p = k * NPOS
        ne = min(NPOS, lout - e0p)
        t0 = e0p - 4
        n_mm = 8 + 2 * ne
        chunks.append((t0, e0p, ne, n_mm))
    wins = []
    i = 0
    for w in _win_schedule(nchunks, ramp, steady_win, tail_ramp):
        wins.append(chunks[i : i + w])
        i += w

    with tile.TileContext(nc) as tc:
        with (
            tc.tile_pool(name="const", bufs=1) as constp,
            tc.tile_pool(name="xd", bufs=xd_bufs) as xdp,
            tc.tile_pool(name="ev", bufs=ev_bufs) as evp,
            tc.tile_pool(name="outp", bufs=ob_bufs) as outp,
            tc.tile_pool(name="psum", bufs=ps_bufs, space=bass.MemorySpace.PSUM) as psp,
        ):
            wt_sb = constp.tile([128, 4 * 256], F8, tag="wt")
            bi_sb = constp.tile([128, 1], F32, tag="bi")
            warm = constp.tile([128, 1], F32, tag="warm")

            def emit_consts():
                nc.sync.dma_start(wt_sb[:], wap[:])
                nc.gpsimd.dma_start(bi_sb[:], bap[:])
                # warm the ACT Identity table before the first activation
                nc.scalar.activation(warm[:], bi_sb[:], AF.Identity, bias=0.0)

            def emit_loads(win):
                s0 = win[0][0] - 3  # position of xd col pair 0
                wspan = max(2 * (t0 - s0) + n_mm for (t0, _, _, n_mm) in win)
                # +4 tile margin for the _slide3 slicing helper; never loaded
                # nor read by the matmul access patterns.
                xd = xdp.tile([128, wspan + 4], F8, tag="xd")
                c0 = 2 * (s0 + PAD)
                nc.sync.dma_start(xd[:, 0:wspan], xxap[:, c0 : c0 + wspan])
                return s0, xd

            def _sched(fracs):
                accs = [0.0] * len(fracs)

                def pick():
                    best, bi_ = None, 0
                    for i, (eng, f) in enumerate(fracs):
                        accs[i] += f
                        if best is None or accs[i] > best:
                            best, bi_ = accs[i], i
                    accs[bi_] -= 1.0
                    return fracs[bi_][0]

                return pick

            pick_heavy = _sched(heavy_fracs)
            pick_c1 = _sched(c1_fracs)
            pick_hc = _sched((("hc", hc_frac), ("dev", 1.0 - hc_frac)))
            pick_st = _sched(st_fracs)
            pick_db = _sched((("drop", dropb_frac), ("keep", 1.0 - dropb_frac)))
            st_engs = {"sp": nc.sync, "act": nc.scalar}

            def st_dma(dst, srcv):
                st_engs[pick_st()].dma_start(dst, srcv)
            # software-pipelined epilogue: c1/c2/store of group i are emitted
            # after group i+lag's matmuls+heavy, so the DVE/Pool queues never
            # head-of-line block on a heavy pass that is still in flight.
            pending = []

            SGW = sg * 2 * NMM
            curS = []  # [(grp, ev, evoff, wtot)] accumulating full pairs

            def emit_mm_group(grp, s0, xd):
                ps = psp.tile([128, 2 * NMM], F32, tag="ps", name="ps")
                for gi, (t0, e0p, ne, n_mm) in enumerate(grp):
                    go = gi * NMM
                    # optionally skip the w_lo correction matmuls (G=1) on a
                    # fraction of chunks: trades ~0.9-1.3% extra rel err for
                    # 2 of 4 matmuls on those chunks
                    nj = 2 if pick_db() == "drop" else 4
                    for j, (g, m) in enumerate(
                        ((0, 0), (0, 1), (1, 0), (1, 1))[:nj]
                    ):
                        base = 2 * (t0 - s0 - 2 * m - 1)
                        lw = wt_sb[
                            :, (2 * g + m) * 256 : (2 * g + m + 1) * 256
                        ].rearrange("p (a q) -> p a q", a=2)
                        nc.tensor.matmul(
                            ps[:, go : go + n_mm],
                            lw,
                            _slide3(xd, base, n_mm),
                            start=(j == 0),
                            stop=(j == nj - 1),
                            perf_mode=DR,
                        )
                return ps

            def emit_heavy(ps, ev, evoff, wtot):
                # heavy: ONE [128, wtot] PSUM->SBUF bf16 pass, bias fused
                # (bias128 is zero on the h=0 rows so it lands once).
                he = pick_heavy()
                if he == "act":
                    nc.scalar.activation(
                        ev[:, evoff : evoff + wtot], ps[:, 0:wtot],
                        AF.Identity, bias=bi_sb[:, 0:1],
                    )
                else:
                    nc.vector.tensor_scalar_add(
                        ev[:, evoff : evoff + wtot], ps[:, 0:wtot],
                        bi_sb[:, 0:1],
                    )

            LASTQ = 2 * (nchunks - 5) * NPOS

            def phase2(sup, force_hc=False):
                ev = sup[0][1]
                wall = sup[-1][2] + sup[-1][3]
                is_tail = 2 * sup[-1][0][-1][1] >= LASTQ
                chunksA = [
                    (evoff // NMM + gi, g)
                    for grp, _, evoff, _ in sup
                    for gi, g in enumerate(grp)
                ]
                if (force_hc or pick_hc() == "hc") and not is_tail:
                    # host-combined super: no on-chip h-add; store the
                    # C_1(+bias) half to out and the shifted C_0 half to aux;
                    # the host adds them (engine work traded for idle DMA).
                    nf = sum(1 for _, g in chunksA if g[2] == NPOS)
                    if nf:
                        qq = 2 * NPOS
                        e0q0 = 2 * sup[0][0][0][1]
                        od = oap[:, e0q0 : e0q0 + nf * qq].rearrange(
                            "p (g q) -> p g q", g=nf
                        )
                        st_dma(od, _pairview(ev, 64, 128, 0, qq, NMM, nf))
                        ad = auxap[:, e0q0 : e0q0 + nf * qq].rearrange(
                            "p (g q) -> p g q", g=nf
                        )
                        st_dma(ad, _pairview(ev, 0, 64, 8, qq, NMM, nf))
                        hc_ranges.append((e0q0, nf * qq))
                    for gidx, (t0, e0p, ne, n_mm) in chunksA[nf:]:
                        qq = 2 * ne
                        st_dma(
                            oap[:, 2 * e0p : 2 * e0p + qq],
                            ev[64:128, gidx * NMM : gidx * NMM + qq],
                        )
                        st_dma(
                            auxap[:, 2 * e0p : 2 * e0p + qq],
                            ev[0:64, gidx * NMM + 8 : gidx * NMM + 8 + qq],
                        )
                        hc_ranges.append((2 * e0p, qq))
                    return
                # cheap1: cross-base copy of the C_1(+bias) half to base
                # partition 0 (2-input ops may not cross SBUF bases); one op
                # spanning the whole super-group's ev tile.
                tm = outp.tile([64, SGW], BF16, tag="tm")
                c1 = "dve" if is_tail else pick_c1()
                if c1 == "dve":
                    nc.vector.tensor_copy(tm[:, 0:wall], ev[64:128, 0:wall])
                elif c1 == "pool":
                    nc.gpsimd.tensor_copy(tm[:, 0:wall], ev[64:128, 0:wall])
                else:
                    nc.scalar.activation(
                        tm[:, 0:wall], ev[64:128, 0:wall], AF.Identity,
                        bias=0.0,
                    )
                # cheap2 (in-place, all-SBUF, base-aligned):
                #   tm[o, (chunk, q)] += ev[o (h=0), (chunk, q+8)]
                chunks_ = chunksA
                nfull = sum(1 for _, g in chunks_ if g[2] == NPOS)
                assert all(g[2] == NPOS for _, g in chunks_[:nfull])
                if nfull:
                    qq = 2 * NPOS
                    o3 = _pairview(tm, 0, 64, 0, qq, NMM, nfull)
                    i0 = _pairview(ev, 0, 64, 8, qq, NMM, nfull)
                    nc.vector.tensor_tensor(o3, o3, i0, ADD)
                for gidx, (t0, e0p, ne, n_mm) in chunks_[nfull:]:
                    qq = 2 * ne
                    tv = tm[:, gidx * NMM : gidx * NMM + qq]
                    nc.vector.tensor_tensor(
                        tv, tv,
                        ev[0:64, gidx * NMM + 8 : gidx * NMM + 8 + qq], ADD,
                    )
                # store: full chunks in one strided DMA; ragged tails alone
                if nfull:
                    qq = 2 * NPOS
                    ost = _pairview(tm, 0, 64, 0, qq, NMM, nfull)
                    e0q0 = 2 * sup[0][0][0][1]
                    od = oap[:, e0q0 : e0q0 + nfull * qq].rearrange(
                        "p (g q) -> p g q", g=nfull
                    )
                    st_dma(od, ost)
                for gidx, (t0, e0p, ne, n_mm) in chunks_[nfull:]:
                    qq = 2 * ne
                    st_dma(
                        oap[:, 2 * e0p : 2 * e0p + qq],
                        tm[:, gidx * NMM : gidx * NMM + qq],
                    )

            def flush_super():
                if curS:
                    pending.append(list(curS))
                    curS.clear()

            ntail = [0]

            def emit_chunks(win, s0, xd, taper=False):
                # pair adjacent full chunks; leftovers go alone
                groups = []
                ci = 0
                while ci < len(win):
                    grp = [win[ci]]
                    ci += 1
                    if (
                        ci < len(win)
                        and grp[0][3] == NMM
                        and win[ci][3] == NMM
                    ):
                        grp.append(win[ci])
                        ci += 1
                    groups.append(grp)
                for grp in groups:
                    ps = emit_mm_group(grp, s0, xd)
                    wtot = (len(grp) - 1) * NMM + grp[-1][3]
                    full_pair = wtot == 2 * NMM
                    if not full_pair:
                        flush_super()
                    ev = (
                        curS[0][1]
                        if curS
                        else evp.tile([128, SGW], BF16, tag="ev")
                    )
                    evoff = curS[-1][2] + 2 * NMM if curS else 0
                    emit_heavy(ps, ev, evoff, wtot)
                    curS.append((grp, ev, evoff, wtot))
                    if not full_pair or len(curS) >= (1 if taper else sg):
                        flush_super()
                    lag = 0 if (taper and taper_lag0) else 1
                    while len(pending) > lag:
                        phase2(pending.pop(0), force_hc=(taper and taper_hc))

            if consts_first:
                emit_consts()
                loaded = [emit_loads(wins[0])]
            else:
                loaded = [emit_loads(wins[0])]
                emit_consts()
            for i, win in enumerate(wins):
                pf = 1 if i == 0 else prefetch
                for j in range(i + 1, min(i + 1 + pf, len(wins))):
                    if j == len(loaded):
                        loaded.append(emit_loads(wins[j]))
                tw = taper_wins if taper_wins is not None else max(
                    1, len(tail_ramp)
                )
                emit_chunks(win, *loaded[i], taper=(i >= len(wins) - tw))
            flush_super()
            for sup in pending:
                phase2(sup, force_hc=taper_hc)
    return xx, wt, bi, out, aux


def pack_x_core(xc, l=L):
    """xc: [2, CIN, l] f32 -> [128, 2*(l+2*PAD)] e4m3: rows 0:64 = e4m3
    hi part, rows 64:128 = e4m3 of the residual; batch-interleaved cols
    (col 2*(t+PAD)+b) with zero margins."""
    x8h = xc.astype(E4M3)
    x8l = (xc - x8h.astype(np.float32)).astype(E4M3)
    arr = np.zeros((128, 2 * (l + 2 * PAD)), dtype=E4M3)
    for r, x8 in ((0, x8h), (64, x8l)):
        v = arr[r : r + CIN, 2 * PAD : 2 * (PAD + l)].reshape(CIN, l, 2)
        v[:, :, 0] = x8[0]
        v[:, :, 1] = x8[1]
    return arr


def pack_weight(weight):
    """[COUT, CIN, KW] f32 -> [128, 1024] e4m3 stationary blocks.
    Block (g, m) cols = (kt, h, o); value w_g[o, c, 4h + 2m + (1-kt)];
    rows = (hl, c) with both hl halves identical."""
    w = np.asarray(weight, dtype=np.float32)
    w8h = w.astype(E4M3)
    w8l = (w - w8h.astype(np.float32)).astype(E4M3)
    blocks = []
    for wg in (w8h, w8l):
        wgf = wg.astype(np.float32)
        for m in range(2):
            blk = np.empty((CIN, 2, 2, COUT), dtype=np.float32)
            for kt in range(2):
                for h in range(2):
                    j = 4 * h + 2 * m + (1 - kt)
                    blk[:, kt, h, :] = wgf[:, :, j].T  # [c, o]
            blocks.append(blk.reshape(CIN, 256))
    half = np.concatenate(blocks, axis=1)  # [64, 1024]
    return np.concatenate([half, half], axis=0).astype(E4M3)


def pack_bias(bias):
    b = np.zeros((128, 1), dtype=np.float32)
    b[64:128, 0] = np.asarray(bias, dtype=np.float32)
    return b


_CACHE = {}


def _compiled():
    if "nc" not in _CACHE:
        nc = bacc.Bacc(
            "TRN2", target_bir_lowering=False, debug=False, num_devices=NCORES
        )
        hc_ranges = []
        handles = build(nc, hc_ranges=hc_ranges)
        nc.compile()
        _CACHE["nc"] = nc
        _CACHE["names"] = [h.name for h in handles]
        _CACHE["hc"] = hc_ranges
    return _CACHE["nc"], _CACHE["names"], _CACHE["hc"]


def run_on_hw(x, weight, bias, trace=False, **kw):
    nc, (xxn, wn, bn, on, an), hc_ranges = _compiled()
    wt_p = pack_weight(weight)
    bi_p = pack_bias(bias)
    x = np.asarray(x, dtype=np.float32)
    in_maps = []
    for k in range(NCORES):
        xx_p = pack_x_core(x[BPC * k : BPC * (k + 1)])
        in_maps.append({xxn: xx_p, wn: wt_p, bn: bi_p})
    res = bass_utils.run_bass_kernel_spmd(
        nc, in_maps, core_ids=list(range(NCORES)), trace=trace, **kw
    )
    outs = []
    for k in range(NCORES):
        oi = np.asarray(res.results[k][on]).astype(np.float32)  # [64, 2*LOUT]
        ai = np.asarray(res.results[k][an])
        for q0, qn in hc_ranges:
            oi[:, q0 : q0 + qn] += ai[:, q0 : q0 + qn].astype(np.float32)
        oi = oi.reshape(COUT, LOUT, 2)
        outs.append(np.stack([oi[:, :, 0], oi[:, :, 1]], axis=0))
    return np.concatenate(outs, axis=0), res


def kernel(x, weight, bias):
    out, _ = run_on_hw(x, weight, bias, trace=False)
    return out
